# revision 16
# baseline (speedup 1.0000x reference)
"""3-layer GCN (ogbn-arxiv scale) on 8 Trainium2 NeuronCores.

Strategy (graph/data parallel):
- Nodes range-sharded across 8 cores (NPC each, 128-padded). Edges
  (incl. self-loops, norm 1/deg) routed to the destination's core, grouped by
  (128-node dst tile, 32768-node source chunk), padded to 128-edge blocks
  (block count per (tile,chunk) = max over cores -> SPMD-uniform program).
- Aggregation per dst tile: one dma_gather per source chunk (int16 local
  indices, 4 SWDGE queues) pulls the source rows; a [dst==j]*norm selection
  matrix (built on DVE) is the matmul lhsT so segment-sum accumulates in PSUM
  node-major. Epilogue matmuls apply W (+BN+ReLU via ACT) feature-major and
  transpose back.
- Layer 1 aggregates x ((A@x)@W1 == A@(x@W1)); layer 3 aggregates h2
  ((A@h2)@W3 == A@(h2@W3)). x, h1, h2 are allgathered (bf16).
"""
import math

import numpy as np
import ml_dtypes

import concourse.bass as bass
import concourse.mybir as mybir
import concourse.tile as tile
from concourse import bacc
from concourse.bass_utils import run_bass_kernel_spmd

NCORES = 8
P = 128
EPS = 1e-5
VCH = 32768
NQ = 4
GROUP_TILES = 4
BF16 = ml_dtypes.bfloat16


def _host_prep(x, edge_index, W1, b1, g1, beta1, m1, v1, W2, b2, g2, beta2, m2, v2,
               W3, b3):
    N, D1 = x.shape
    D2 = W1.shape[1]
    DO = W3.shape[1]
    NPC = int(math.ceil(N / (NCORES * P))) * P
    NT = NPC // P
    NF = NCORES * NPC
    NCH = int(math.ceil(NF / VCH))

    src = np.asarray(edge_index[0], dtype=np.int64)
    dst = np.asarray(edge_index[1], dtype=np.int64)
    deg = (np.bincount(dst, minlength=N).astype(np.float64) + 1.0)
    dinv = (1.0 / np.sqrt(deg)).astype(np.float32)
    en = dinv[src] * dinv[dst]

    all_src = np.concatenate([src, np.arange(N, dtype=np.int64)])
    all_dst = np.concatenate([dst, np.arange(N, dtype=np.int64)])
    all_nrm = np.concatenate([en, (1.0 / deg).astype(np.float32)])

    core = all_dst // NPC
    tl = (all_dst % NPC) // P
    ch = all_src // VCH
    # sort by (core, tile, chunk)
    key = (core * NT + tl) * NCH + ch
    order = np.argsort(key, kind="stable")
    s_src = all_src[order]
    s_dst = all_dst[order]
    s_nrm = all_nrm[order].astype(np.float32)
    s_key = key[order]
    s_core = s_key // (NT * NCH)
    s_tc = s_key % (NT * NCH)          # tile*NCH + chunk
    dl = (s_dst % P).astype(np.float32)
    s_loc = (s_src % VCH).astype(np.int64)

    ngrp = NCORES * NT * NCH
    grp_start = np.searchsorted(s_key, np.arange(ngrp), side="left")
    cnt = np.diff(np.append(grp_start, len(s_key)))  # [ngrp]
    cnt3 = cnt.reshape(NCORES, NT * NCH)
    kpad_flat = np.ceil(cnt3.max(axis=0) / P).astype(np.int64)  # [NT*NCH]
    KPAD = kpad_flat.reshape(NT, NCH)

    # block bases: blocks laid out t-major then chunk (for sel/dloc/nrm)
    blk_base = np.zeros(NT * NCH + 1, np.int64)
    np.cumsum(kpad_flat, out=blk_base[1:])
    total_blocks = int(blk_base[-1])

    # slot position within each (core, t, c) group
    r = np.arange(len(s_key)) - grp_start[s_key]
    pp = r % P
    blk = blk_base[s_tc] + r // P          # global block id (tile-major)

    # ---- group-major layout for gather indices: (g, c, tloc, k) ----
    GT = GROUP_TILES
    NG = (NT + GT - 1) // GT
    t_of = np.arange(NT * NCH) // NCH
    c_of = np.arange(NT * NCH) % NCH
    gm_rank = ((t_of // GT) * NCH + c_of) * GT + (t_of % GT)  # rank of (t,c)
    gm_order = np.argsort(gm_rank)          # (t,c) id sorted by group-major
    # col8 base per (t,c) in group-major concat
    gcol8 = np.zeros(NT * NCH + 1, np.int64)
    np.cumsum(kpad_flat[gm_order] * 8, out=gcol8[1:])
    gcol8_of = np.zeros(NT * NCH, np.int64)
    gcol8_of[gm_order] = gcol8[:-1]
    total_cols8 = int(gcol8[-1])
    iwcol = gcol8_of[s_tc] + r // 16
    iwrow = r % 16

    dloc_a = np.full((NCORES, P, total_blocks), 255.0, np.float32)
    nrm_a = np.zeros((NCORES, P, total_blocks), np.float32)
    idxw = np.zeros((NCORES, 16, total_cols8), np.int16)
    dloc_a[s_core, pp, blk] = dl
    nrm_a[s_core, pp, blk] = s_nrm
    idxw[s_core, iwrow, iwcol] = s_loc.astype(np.int16)
    idxw_full = np.tile(idxw, (1, 8, 1))   # replicate to 128 partitions

    xp = np.zeros((NF, D1), np.float32)
    xp[:N] = x
    x_bf = xp.astype(BF16)

    def wdev(W):
        Din, Dout = W.shape
        return (W.reshape(Din // P, P, Dout).transpose(1, 0, 2)
                .reshape(P, (Din // P) * Dout).astype(BF16))

    sc1 = (g1 / np.sqrt(v1 + EPS)).astype(np.float32)
    bi1 = (beta1 - m1 * sc1 + b1 * sc1).astype(np.float32)
    sc2 = (g2 / np.sqrt(v2 + EPS)).astype(np.float32)
    bi2 = (beta2 - m2 * sc2 + b2 * sc2).astype(np.float32)
    nh = D2 // P
    sbn = np.zeros((P, 4 * nh), np.float32)
    for oh in range(nh):
        sbn[:, 4 * oh + 0] = sc1[oh * P:(oh + 1) * P]
        sbn[:, 4 * oh + 1] = bi1[oh * P:(oh + 1) * P]
        sbn[:, 4 * oh + 2] = sc2[oh * P:(oh + 1) * P]
        sbn[:, 4 * oh + 3] = bi2[oh * P:(oh + 1) * P]

    b3c = np.zeros((P, 1), np.float32)
    b3c[:DO, 0] = b3.astype(np.float32)
    Jm = np.broadcast_to(np.arange(P, dtype=np.float32), (P, P)).astype(BF16)
    ident = np.eye(P, dtype=np.float32).astype(BF16)

    in_maps = []
    for c in range(NCORES):
        in_maps.append({
            "xsh": x_bf[c * NPC:(c + 1) * NPC],
            "idxw": idxw_full[c],
            "dloc": dloc_a[c].astype(BF16),
            "nrm": nrm_a[c].astype(BF16),
            "W1d": wdev(W1),
            "W2d": wdev(W2),
            "W3d": W3.astype(BF16),
            "sbn": sbn,
            "b3c": b3c,
            "J": Jm,
            "ident": ident,
        })
    dims = dict(N=N, NPC=NPC, NT=NT, D1=D1, D2=D2, DO=DO, NCH=NCH,
                KPAD=tuple(map(tuple, KPAD.tolist())),
                total_blocks=total_blocks, total_cols8=total_cols8)
    return in_maps, dims


def _build(dims):
    N, NPC, NT = dims["N"], dims["NPC"], dims["NT"]
    D1, D2, DO = dims["D1"], dims["D2"], dims["DO"]
    NCH = dims["NCH"]
    KPAD = dims["KPAD"]
    total_blocks = dims["total_blocks"]
    total_cols8 = dims["total_cols8"]
    NF = NCORES * NPC
    nh = D2 // P
    GT = GROUP_TILES
    NG = (NT + GT - 1) // GT
    bf = mybir.dt.bfloat16
    f32 = mybir.dt.float32
    debug = dims.get("debug", 0)

    # tile-major block bases (sel/dloc/nrm)
    blk_base = [[0] * (NCH + 1) for _ in range(NT)]
    acc_b = 0
    for t in range(NT):
        for c in range(NCH):
            blk_base[t][c] = acc_b
            acc_b += KPAD[t][c]
        blk_base[t][NCH] = acc_b
    assert acc_b == total_blocks
    max_ktot = max(blk_base[t][NCH] - blk_base[t][0] for t in range(NT))

    # group-major col8 bases (idxw) + sub-block offsets within (g,c) gathers
    gcol8 = {}     # (g,c) -> col8 base of the (g,c) call
    gsub = {}      # (t,c) -> block offset within its (g,c) gathered tile
    gnum = {}      # (g,c) -> total blocks in the call
    acc_c8 = 0
    for g in range(NG):
        tiles = list(range(g * GT, min((g + 1) * GT, NT)))
        for c in range(NCH):
            gcol8[(g, c)] = acc_c8
            off = 0
            for t in tiles:
                gsub[(t, c)] = off
                off += KPAD[t][c]
            gnum[(g, c)] = off
            acc_c8 += off * 8
    assert acc_c8 == total_cols8
    gmax = [max(gnum[(g, c)] for g in range(NG)) for c in range(NCH)]
    gcols = {g: sum(gnum[(g, c)] for c in range(NCH)) * 8 for g in range(NG)}
    max_gcols = max(gcols.values())

    nc = bacc.Bacc("TRN2", target_bir_lowering=False, debug=False,
                   num_devices=NCORES, num_swdge_queues=NQ)
    xsh_e = nc.dram_tensor("xsh", [NPC, D1], bf, kind="ExternalInput").ap()
    idxw_e = nc.dram_tensor("idxw", [P, total_cols8], mybir.dt.int16, kind="ExternalInput").ap()
    dloc_e = nc.dram_tensor("dloc", [P, total_blocks], bf, kind="ExternalInput").ap()
    nrm_e = nc.dram_tensor("nrm", [P, total_blocks], bf, kind="ExternalInput").ap()
    W1d_e = nc.dram_tensor("W1d", [P, (D1 // P) * D2], bf, kind="ExternalInput").ap()
    W2d_e = nc.dram_tensor("W2d", [P, (D2 // P) * D2], bf, kind="ExternalInput").ap()
    W3d_e = nc.dram_tensor("W3d", [D2, DO], bf, kind="ExternalInput").ap()
    sbn_e = nc.dram_tensor("sbn", [P, 4 * nh], f32, kind="ExternalInput").ap()
    b3c_e = nc.dram_tensor("b3c", [P, 1], f32, kind="ExternalInput").ap()
    J_e = nc.dram_tensor("J", [P, P], bf, kind="ExternalInput").ap()
    id_e = nc.dram_tensor("ident", [P, P], bf, kind="ExternalInput").ap()

    outp_e = nc.dram_tensor("outp", [NPC, DO], f32, kind="ExternalOutput").ap()
    emb_e = nc.dram_tensor("emb", [NPC, D2], f32, kind="ExternalOutput").ap()

    xshb = nc.dram_tensor("xshb", [NPC, D1], bf).ap()
    xfull = nc.dram_tensor("xfull", [NF, D1], bf).ap()
    h1sh = nc.dram_tensor("h1sh", [NPC, D2], bf).ap()
    h1full = nc.dram_tensor("h1full", [NF, D2], bf).ap()
    h2sh = nc.dram_tensor("h2sh", [NPC, D2], bf).ap()
    h2full = nc.dram_tensor("h2full", [NF, D2], bf).ap()

    RG = [list(range(NCORES))]
    qctr = [0]

    with tile.TileContext(nc) as tc:
        with (
            tc.tile_pool(name="const", bufs=1) as const,
            tc.tile_pool(name="selp", bufs=3) as selp,
            tc.tile_pool(name="epi", bufs=2) as epi,
        ):
            # ---- resident constants ----
            dloc_t = const.tile([P, total_blocks], bf)
            nc.sync.dma_start(out=dloc_t[:], in_=dloc_e[:, :])
            nrm_t = const.tile([P, total_blocks], bf)
            nc.sync.dma_start(out=nrm_t[:], in_=nrm_e[:, :])
            W1_t = const.tile([P, (D1 // P) * D2], bf)
            nc.sync.dma_start(out=W1_t[:], in_=W1d_e[:, :])
            W2_t = const.tile([P, (D2 // P) * D2], bf)
            nc.sync.dma_start(out=W2_t[:], in_=W2d_e[:, :])
            W3_t = []
            for ih in range(D2 // P):
                w3 = const.tile([P, DO], bf, tag=f"w3_{ih}")
                nc.sync.dma_start(out=w3[:], in_=W3d_e[ih * P:(ih + 1) * P, :])
                W3_t.append(w3)
            sbn_t = const.tile([P, 4 * nh], f32)
            nc.sync.dma_start(out=sbn_t[:], in_=sbn_e[:, :])
            b3c_t = const.tile([P, 1], f32)
            nc.sync.dma_start(out=b3c_t[:], in_=b3c_e[:, :])
            J_t = const.tile([P, P], bf)
            nc.sync.dma_start(out=J_t[:], in_=J_e[:, :])
            id_t = const.tile([P, P], bf)
            nc.sync.dma_start(out=id_t[:], in_=id_e[:, :])

            # ---- x allgather ----
            nc.sync.dma_start(out=xshb[:, :], in_=xsh_e[:, :])
            nc.gpsimd.collective_compute(
                "AllGather", mybir.AluOpType.bypass, replica_groups=RG,
                ins=[xshb.opt()], outs=[xfull.opt()],
            )

            def build_sel(t):
                ktot = blk_base[t][NCH] - blk_base[t][0]
                b0 = blk_base[t][0]
                sel = selp.tile([P, max_ktot * P], bf, tag="sel")
                sel3 = sel[:, :ktot * P].rearrange("p (k j) -> p k j", k=ktot)
                nc.vector.tensor_tensor(
                    out=sel3,
                    in0=dloc_t[:, b0:b0 + ktot].unsqueeze(2).broadcast_to([P, ktot, P]),
                    in1=J_t[:].unsqueeze(1).broadcast_to([P, ktot, P]),
                    op=mybir.AluOpType.is_equal,
                ).annotate("sel_eq")
                nc.vector.tensor_tensor(
                    out=sel3, in0=sel3,
                    in1=nrm_t[:, b0:b0 + ktot].unsqueeze(2).broadcast_to([P, ktot, P]),
                    op=mybir.AluOpType.mult,
                ).annotate("sel_mul")
                return sel

            def gather_group(g, table, D, tag, stream):
                """One dma_gather per chunk for the whole tile group."""
                cg = gcols[g]
                ixt = stream.tile([P, max_gcols], mybir.dt.int16, tag=f"ix{tag}")
                nc.sync.dma_start(
                    out=ixt[:, :cg],
                    in_=idxw_e[:, gcol8[(g, 0)]:gcol8[(g, 0)] + cg],
                ).annotate(f"idx_{tag}")
                gts = []
                for c in range(NCH):
                    nb = gnum[(g, c)]
                    if nb == 0:
                        gts.append(None)
                        continue
                    gt = stream.tile([P, gmax[c] * D], bf, tag=f"g{tag}{c}")
                    g3 = gt[:, :nb * D].rearrange("p (k d) -> p k d", d=D)
                    rows = min(VCH, NF - c * VCH)
                    lb = gcol8[(g, c)] - gcol8[(g, 0)]
                    nc.gpsimd.dma_gather(
                        out_ap=g3,
                        in_ap=table[c * VCH:c * VCH + rows, :],
                        idxs_ap=ixt[:, lb:lb + nb * 8],
                        num_idxs=nb * P, num_idxs_reg=nb * P, elem_size=D,
                        queue_num=qctr[0] % NQ, single_packet=False,
                    ).annotate(f"gth_{tag}")
                    qctr[0] += 1
                    gts.append(gt)
                return gts

            def agg(ps_s, sel, gts, t, g, D):
                ktot = blk_base[t][NCH] - blk_base[t][0]
                bi = 0
                for c in range(NCH):
                    kc = KPAD[t][c]
                    sub = gsub[(t, c)]
                    for k in range(kc):
                        nc.tensor.matmul(
                            out=ps_s[:],
                            lhsT=sel[:, bi * P:(bi + 1) * P],
                            rhs=gts[c][:, (sub + k) * D:(sub + k + 1) * D],
                            start=(bi == 0), stop=(bi == ktot - 1),
                        ).annotate("aggmm")
                        bi += 1

            # ================= Layer 1 =================
            with (
                tc.tile_pool(name="ps1", bufs=2, space="PSUM") as ps,
                tc.tile_pool(name="ps1e", bufs=1, space="PSUM") as pse,
                tc.tile_pool(name="stream1", bufs=2) as stream,
            ):
                for g in range(NG):
                    gts = gather_group(g, xfull, D1, "a", stream)
                    for t in range(g * GT, min((g + 1) * GT, NT)):
                        sel = build_sel(t)
                        ps_s = ps.tile([P, D1], f32, tag="s")
                        agg(ps_s, sel, gts, t, g, D1)
                        s_bf = epi.tile([P, D1], bf, tag="sbf")
                        nc.vector.tensor_copy(out=s_bf[:], in_=ps_s[:])
                        ps_t = pse.tile([P, D1], bf, tag="st")
                        nc.tensor.transpose(out=ps_t[:], in_=s_bf[:], identity=id_t[:])
                        sT = epi.tile([P, D1], bf, tag="sT")
                        nc.vector.tensor_copy(out=sT[:], in_=ps_t[:])
                        hT = epi.tile([P, D2], bf, tag="hT")
                        for oh in range(nh):
                            ps_h = pse.tile([P, P], f32, tag=f"h{oh}")
                            nc.tensor.matmul(
                                out=ps_h[:], lhsT=W1_t[:, oh * P:(oh + 1) * P],
                                rhs=sT[:], start=True, stop=True,
                            )
                            nc.scalar.activation(
                                out=hT[:, oh * P:(oh + 1) * P], in_=ps_h[:],
                                func=mybir.ActivationFunctionType.Relu,
                                scale=sbn_t[:, 4 * oh:4 * oh + 1],
                                bias=sbn_t[:, 4 * oh + 1:4 * oh + 2],
                            )
                        ps_nm = pse.tile([P, D2], bf, tag="nm")
                        for oh in range(nh):
                            nc.tensor.transpose(
                                out=ps_nm[:, oh * P:(oh + 1) * P],
                                in_=hT[:, oh * P:(oh + 1) * P], identity=id_t[:])
                        h_nm = epi.tile([P, D2], bf, tag="hnm")
                        nc.vector.tensor_copy(out=h_nm[:], in_=ps_nm[:])
                        nc.sync.dma_start(out=h1sh[t * P:(t + 1) * P, :], in_=h_nm[:])

            nc.gpsimd.collective_compute(
                "AllGather", mybir.AluOpType.bypass, replica_groups=RG,
                ins=[h1sh.opt()], outs=[h1full.opt()],
            )

            # ================= Layer 2 =================
            with (
                tc.tile_pool(name="ps2", bufs=2, space="PSUM") as ps,
                tc.tile_pool(name="ps2e", bufs=1, space="PSUM") as pse,
                tc.tile_pool(name="stream2", bufs=2) as stream,
            ):
                for g in range(NG):
                    gts = gather_group(g, h1full, D2, "b", stream)
                    for t in range(g * GT, min((g + 1) * GT, NT)):
                        sel = build_sel(t)
                        ps_s = ps.tile([P, D2], f32, tag="s")
                        agg(ps_s, sel, gts, t, g, D2)
                        s_bf = epi.tile([P, D2], bf, tag="sbf2")
                        nc.vector.tensor_copy(out=s_bf[:], in_=ps_s[:])
                        ps_t = pse.tile([P, D2], bf, tag="st")
                        for ih in range(D2 // P):
                            nc.tensor.transpose(
                                out=ps_t[:, ih * P:(ih + 1) * P],
                                in_=s_bf[:, ih * P:(ih + 1) * P], identity=id_t[:])
                        sT = epi.tile([P, D2], bf, tag="sT2")
                        nc.vector.tensor_copy(out=sT[:], in_=ps_t[:])
                        hT = epi.tile([P, D2], bf, tag="hT2")
                        for oh in range(nh):
                            ps_h = pse.tile([P, P], f32, tag=f"h{oh}")
                            for ih in range(D2 // P):
                                nc.tensor.matmul(
                                    out=ps_h[:],
                                    lhsT=W2_t[:, ih * D2 + oh * P: ih * D2 + (oh + 1) * P],
                                    rhs=sT[:, ih * P:(ih + 1) * P],
                                    start=(ih == 0), stop=(ih == D2 // P - 1),
                                )
                            nc.scalar.activation(
                                out=hT[:, oh * P:(oh + 1) * P], in_=ps_h[:],
                                func=mybir.ActivationFunctionType.Relu,
                                scale=sbn_t[:, 4 * oh + 2:4 * oh + 3],
                                bias=sbn_t[:, 4 * oh + 3:4 * oh + 4],
                            )
                        ps_nm = pse.tile([P, D2], bf, tag="nm")
                        for oh in range(nh):
                            nc.tensor.transpose(
                                out=ps_nm[:, oh * P:(oh + 1) * P],
                                in_=hT[:, oh * P:(oh + 1) * P], identity=id_t[:])
                        h_nm = epi.tile([P, D2], bf, tag="hnm2")
                        nc.vector.tensor_copy(out=h_nm[:], in_=ps_nm[:])
                        nc.sync.dma_start(out=h2sh[t * P:(t + 1) * P, :], in_=h_nm[:])
                        emb_sb = epi.tile([P, D2], f32, tag="embsb")
                        nc.vector.tensor_copy(out=emb_sb[:], in_=h_nm[:])
                        nc.sync.dma_start(out=emb_e[t * P:(t + 1) * P, :], in_=emb_sb[:])

            nc.gpsimd.collective_compute(
                "AllGather", mybir.AluOpType.bypass, replica_groups=RG,
                ins=[h2sh.opt()], outs=[h2full.opt()],
            )

            # ================= Layer 3 =================
            with (
                tc.tile_pool(name="ps3", bufs=2, space="PSUM") as ps,
                tc.tile_pool(name="ps3e", bufs=1, space="PSUM") as pse,
                tc.tile_pool(name="stream3", bufs=2) as stream,
            ):
                for g in range(NG):
                    gts = gather_group(g, h2full, D2, "c", stream)
                    for t in range(g * GT, min((g + 1) * GT, NT)):
                        sel = build_sel(t)
                        ps_s = ps.tile([P, D2], f32, tag="s3")
                        agg(ps_s, sel, gts, t, g, D2)
                        s_bf = epi.tile([P, D2], bf, tag="sbf3")
                        nc.vector.tensor_copy(out=s_bf[:], in_=ps_s[:])
                        ps_t = pse.tile([P, D2], bf, tag="st3")
                        for ih in range(D2 // P):
                            nc.tensor.transpose(
                                out=ps_t[:, ih * P:(ih + 1) * P],
                                in_=s_bf[:, ih * P:(ih + 1) * P], identity=id_t[:])
                        sT = epi.tile([P, D2], bf, tag="sT3")
                        nc.vector.tensor_copy(out=sT[:], in_=ps_t[:])
                        ps_o = pse.tile([P, P], f32, tag="o")
                        for ih in range(D2 // P):
                            nc.tensor.matmul(
                                out=ps_o[:DO, :],
                                lhsT=W3_t[ih][:],
                                rhs=sT[:, ih * P:(ih + 1) * P],
                                start=(ih == 0), stop=(ih == D2 // P - 1),
                            )
                        o_bf = epi.tile([P, P], bf, tag="obf")
                        nc.vector.tensor_tensor(
                            out=o_bf[:DO, :], in0=ps_o[:DO, :],
                            in1=b3c_t[:DO, :1].to_broadcast([DO, P]),
                            op=mybir.AluOpType.add)
                        ps_on = pse.tile([P, DO], bf, tag="on")
                        nc.tensor.transpose(
                            out=ps_on[:], in_=o_bf[:DO, :], identity=id_t[:DO, :DO])
                        o_sb = epi.tile([P, DO], f32, tag="osb")
                        nc.vector.tensor_copy(out=o_sb[:], in_=ps_on[:])
                        mx = epi.tile([P, 1], f32, tag="mx")
                        nc.vector.tensor_reduce(
                            out=mx[:], in_=o_sb[:], axis=mybir.AxisListType.X,
                            op=mybir.AluOpType.max)
                        nmx = epi.tile([P, 1], f32, tag="nmx")
                        nc.vector.tensor_scalar_mul(out=nmx[:], in0=mx[:], scalar1=-1.0)
                        eo = epi.tile([P, DO], f32, tag="eo")
                        sm = epi.tile([P, 1], f32, tag="sm")
                        nc.scalar.activation(
                            out=eo[:], in_=o_sb[:],
                            func=mybir.ActivationFunctionType.Exp,
                            bias=nmx[:, :1], scale=1.0, accum_out=sm[:])
                        lnsm = epi.tile([P, 1], f32, tag="lnsm")
                        nc.scalar.activation(
                            out=lnsm[:], in_=sm[:],
                            func=mybir.ActivationFunctionType.Ln)
                        lse = epi.tile([P, 1], f32, tag="lse")
                        nc.vector.tensor_tensor(
                            out=lse[:], in0=mx[:], in1=lnsm[:],
                            op=mybir.AluOpType.add)
                        of = epi.tile([P, DO], f32, tag="of")
                        nc.vector.tensor_tensor(
                            out=of[:], in0=o_sb[:],
                            in1=lse[:].to_broadcast([P, DO]),
                            op=mybir.AluOpType.subtract)
                        nc.sync.dma_start(out=outp_e[t * P:(t + 1) * P, :], in_=of[:])

    nc.compile()
    return nc


_CACHE = {}


def _get_program(dims):
    key = (dims["N"], dims["NPC"], dims["NT"], dims["D1"], dims["D2"],
           dims["DO"], dims["NCH"], dims["KPAD"], dims.get("debug", 0))
    if key not in _CACHE:
        _CACHE[key] = _build(dims)
    return _CACHE[key]


def run_impl(inputs, trace=False):
    in_maps, dims = _host_prep(**inputs)
    nc = _get_program(dims)
    res = run_bass_kernel_spmd(nc, in_maps, core_ids=list(range(NCORES)),
                               trace=trace)
    N = dims["N"]
    out = np.concatenate([r["outp"] for r in res.results], axis=0)[:N]
    emb = np.concatenate([r["emb"] for r in res.results], axis=0)[:N]
    return (out.astype(np.float32), emb.astype(np.float32)), res


def kernel(**inputs):
    (out, emb), _ = run_impl(inputs, trace=False)
    return out, emb


# revision 19
# speedup vs baseline: 1.2587x; 1.2587x over previous
"""3-layer GCN (ogbn-arxiv scale) on 8 Trainium2 NeuronCores.

Strategy (graph/data parallel):
- Nodes range-sharded across 8 cores (NPC each, 128-padded). Edges
  (incl. self-loops, norm 1/deg) routed to the destination's core, grouped by
  (128-node dst tile, 32768-node source chunk), padded to 128-edge blocks
  (block count per (tile,chunk) = max over cores -> SPMD-uniform program).
- Aggregation per dst tile: one dma_gather per source chunk (int16 local
  indices, 4 SWDGE queues) pulls the source rows; a [dst==j]*norm selection
  matrix (built on DVE) is the matmul lhsT so segment-sum accumulates in PSUM
  node-major. Epilogue matmuls apply W (+BN+ReLU via ACT) feature-major and
  transpose back.
- Layer 1 aggregates x ((A@x)@W1 == A@(x@W1)); layer 3 aggregates h2
  ((A@h2)@W3 == A@(h2@W3)). x, h1, h2 are allgathered (bf16).
"""
import math

import numpy as np
import ml_dtypes

import concourse.bass as bass
import concourse.mybir as mybir
import concourse.tile as tile
from concourse import bacc
from concourse.bass_utils import run_bass_kernel_spmd

NCORES = 8
P = 128
EPS = 1e-5
VCH = 32768
NQ = 4
GROUP_TILES = 4
BF16 = ml_dtypes.bfloat16


def _host_prep(x, edge_index, W1, b1, g1, beta1, m1, v1, W2, b2, g2, beta2, m2, v2,
               W3, b3):
    N, D1 = x.shape
    D2 = W1.shape[1]
    DO = W3.shape[1]
    NPC = int(math.ceil(N / (NCORES * P))) * P
    NT = NPC // P
    NF = NCORES * NPC
    NCH = int(math.ceil(NF / VCH))

    src = np.asarray(edge_index[0], dtype=np.int64)
    dst = np.asarray(edge_index[1], dtype=np.int64)
    deg = (np.bincount(dst, minlength=N).astype(np.float64) + 1.0)
    dinv = (1.0 / np.sqrt(deg)).astype(np.float32)
    en = dinv[src] * dinv[dst]

    all_src = np.concatenate([src, np.arange(N, dtype=np.int64)])
    all_dst = np.concatenate([dst, np.arange(N, dtype=np.int64)])
    all_nrm = np.concatenate([en, (1.0 / deg).astype(np.float32)])

    core = all_dst // NPC
    tl = (all_dst % NPC) // P
    # quarter-major table numbering: table row of node (c, r) =
    # qbase(q(r)) + c*qrows(q) + (r - qr0(q)); quarters tile-aligned.
    qb = [(NT * a // 4) * P for a in range(5)]
    m = np.zeros(NCORES * NPC, np.int64)
    qbase = 0
    for a in range(4):
        r0, r1 = qb[a], qb[a + 1]
        qr = r1 - r0
        for c in range(NCORES):
            m[c * NPC + r0:c * NPC + r1] = np.arange(
                qbase + c * qr, qbase + c * qr + qr)
        qbase += NCORES * qr
    tbl_src = m[all_src]
    ch = tbl_src // VCH
    # sort by (core, tile, chunk)
    key = (core * NT + tl) * NCH + ch
    order = np.argsort(key, kind="stable")
    s_src = all_src[order]
    s_dst = all_dst[order]
    s_nrm = all_nrm[order].astype(np.float32)
    s_key = key[order]
    s_core = s_key // (NT * NCH)
    s_tc = s_key % (NT * NCH)          # tile*NCH + chunk
    dl = (s_dst % P).astype(np.float32)
    s_loc = (m[s_src] % VCH).astype(np.int64)

    ngrp = NCORES * NT * NCH
    grp_start = np.searchsorted(s_key, np.arange(ngrp), side="left")
    cnt = np.diff(np.append(grp_start, len(s_key)))  # [ngrp]
    cnt3 = cnt.reshape(NCORES, NT * NCH)
    kpad_flat = np.ceil(cnt3.max(axis=0) / P).astype(np.int64)  # [NT*NCH]
    KPAD = kpad_flat.reshape(NT, NCH)

    # block bases: blocks laid out t-major then chunk (for sel/dloc/nrm)
    blk_base = np.zeros(NT * NCH + 1, np.int64)
    np.cumsum(kpad_flat, out=blk_base[1:])
    total_blocks = int(blk_base[-1])

    # slot position within each (core, t, c) group
    r = np.arange(len(s_key)) - grp_start[s_key]
    pp = r % P
    blk = blk_base[s_tc] + r // P          # global block id (tile-major)

    # ---- group-major layout for gather indices: (g, c, tloc, k) ----
    GT = GROUP_TILES
    NG = (NT + GT - 1) // GT
    t_of = np.arange(NT * NCH) // NCH
    c_of = np.arange(NT * NCH) % NCH
    gm_rank = ((t_of // GT) * NCH + c_of) * GT + (t_of % GT)  # rank of (t,c)
    gm_order = np.argsort(gm_rank)          # (t,c) id sorted by group-major
    # col8 base per (t,c) in group-major concat
    gcol8 = np.zeros(NT * NCH + 1, np.int64)
    np.cumsum(kpad_flat[gm_order] * 8, out=gcol8[1:])
    gcol8_of = np.zeros(NT * NCH, np.int64)
    gcol8_of[gm_order] = gcol8[:-1]
    total_cols8 = int(gcol8[-1])
    iwcol = gcol8_of[s_tc] + r // 16
    iwrow = r % 16

    dloc_a = np.full((NCORES, P, total_blocks), 255.0, np.float32)
    nrm_a = np.zeros((NCORES, P, total_blocks), np.float32)
    idxw = np.zeros((NCORES, 16, total_cols8), np.int16)
    dloc_a[s_core, pp, blk] = dl
    nrm_a[s_core, pp, blk] = s_nrm
    idxw[s_core, iwrow, iwcol] = s_loc.astype(np.int16)
    idxw_full = np.tile(idxw, (1, 8, 1))   # replicate to 128 partitions

    xp = np.zeros((NF, D1), np.float32)
    xp[:N] = x
    x_bf = xp.astype(BF16)

    def wdev(W):
        Din, Dout = W.shape
        return (W.reshape(Din // P, P, Dout).transpose(1, 0, 2)
                .reshape(P, (Din // P) * Dout).astype(BF16))

    sc1 = (g1 / np.sqrt(v1 + EPS)).astype(np.float32)
    bi1 = (beta1 - m1 * sc1 + b1 * sc1).astype(np.float32)
    sc2 = (g2 / np.sqrt(v2 + EPS)).astype(np.float32)
    bi2 = (beta2 - m2 * sc2 + b2 * sc2).astype(np.float32)
    nh = D2 // P
    sbn = np.zeros((P, 4 * nh), np.float32)
    for oh in range(nh):
        sbn[:, 4 * oh + 0] = sc1[oh * P:(oh + 1) * P]
        sbn[:, 4 * oh + 1] = bi1[oh * P:(oh + 1) * P]
        sbn[:, 4 * oh + 2] = sc2[oh * P:(oh + 1) * P]
        sbn[:, 4 * oh + 3] = bi2[oh * P:(oh + 1) * P]

    b3c = np.zeros((P, 1), np.float32)
    b3c[:DO, 0] = b3.astype(np.float32)
    Jm = np.broadcast_to(np.arange(P, dtype=np.float32), (P, P)).astype(BF16)
    ident = np.eye(P, dtype=np.float32).astype(BF16)

    in_maps = []
    for c in range(NCORES):
        in_maps.append({
            "xsh": x_bf[c * NPC:(c + 1) * NPC],
            "idxw": idxw_full[c],
            "dloc": dloc_a[c].astype(BF16),
            "nrm": nrm_a[c].astype(BF16),
            "W1d": wdev(W1),
            "W2d": wdev(W2),
            "W3d": W3.astype(BF16),
            "sbn": sbn,
            "b3c": b3c,
            "J": Jm,
            "ident": ident,
        })
    dims = dict(N=N, NPC=NPC, NT=NT, D1=D1, D2=D2, DO=DO, NCH=NCH,
                KPAD=tuple(map(tuple, KPAD.tolist())),
                total_blocks=total_blocks, total_cols8=total_cols8)
    return in_maps, dims


def _build(dims):
    N, NPC, NT = dims["N"], dims["NPC"], dims["NT"]
    D1, D2, DO = dims["D1"], dims["D2"], dims["DO"]
    NCH = dims["NCH"]
    KPAD = dims["KPAD"]
    total_blocks = dims["total_blocks"]
    total_cols8 = dims["total_cols8"]
    NF = NCORES * NPC
    nh = D2 // P
    GT = GROUP_TILES
    NG = (NT + GT - 1) // GT
    bf = mybir.dt.bfloat16
    f32 = mybir.dt.float32
    debug = dims.get("debug", 0)

    # tile-major block bases (sel/dloc/nrm)
    blk_base = [[0] * (NCH + 1) for _ in range(NT)]
    acc_b = 0
    for t in range(NT):
        for c in range(NCH):
            blk_base[t][c] = acc_b
            acc_b += KPAD[t][c]
        blk_base[t][NCH] = acc_b
    assert acc_b == total_blocks
    max_ktot = max(blk_base[t][NCH] - blk_base[t][0] for t in range(NT))

    # group-major col8 bases (idxw) + sub-block offsets within (g,c) gathers
    gcol8 = {}     # (g,c) -> col8 base of the (g,c) call
    gsub = {}      # (t,c) -> block offset within its (g,c) gathered tile
    gnum = {}      # (g,c) -> total blocks in the call
    acc_c8 = 0
    for g in range(NG):
        tiles = list(range(g * GT, min((g + 1) * GT, NT)))
        for c in range(NCH):
            gcol8[(g, c)] = acc_c8
            off = 0
            for t in tiles:
                gsub[(t, c)] = off
                off += KPAD[t][c]
            gnum[(g, c)] = off
            acc_c8 += off * 8
    assert acc_c8 == total_cols8
    gmax = [max(gnum[(g, c)] for g in range(NG)) for c in range(NCH)]
    gcols = {g: sum(gnum[(g, c)] for c in range(NCH)) * 8 for g in range(NG)}
    max_gcols = max(gcols.values())

    nc = bacc.Bacc("TRN2", target_bir_lowering=False, debug=False,
                   num_devices=NCORES, num_swdge_queues=NQ)
    xsh_e = nc.dram_tensor("xsh", [NPC, D1], bf, kind="ExternalInput").ap()
    idxw_e = nc.dram_tensor("idxw", [P, total_cols8], mybir.dt.int16, kind="ExternalInput").ap()
    dloc_e = nc.dram_tensor("dloc", [P, total_blocks], bf, kind="ExternalInput").ap()
    nrm_e = nc.dram_tensor("nrm", [P, total_blocks], bf, kind="ExternalInput").ap()
    W1d_e = nc.dram_tensor("W1d", [P, (D1 // P) * D2], bf, kind="ExternalInput").ap()
    W2d_e = nc.dram_tensor("W2d", [P, (D2 // P) * D2], bf, kind="ExternalInput").ap()
    W3d_e = nc.dram_tensor("W3d", [D2, DO], bf, kind="ExternalInput").ap()
    sbn_e = nc.dram_tensor("sbn", [P, 4 * nh], f32, kind="ExternalInput").ap()
    b3c_e = nc.dram_tensor("b3c", [P, 1], f32, kind="ExternalInput").ap()
    J_e = nc.dram_tensor("J", [P, P], bf, kind="ExternalInput").ap()
    id_e = nc.dram_tensor("ident", [P, P], bf, kind="ExternalInput").ap()

    outp_e = nc.dram_tensor("outp", [NPC, DO], f32, kind="ExternalOutput").ap()
    emb_e = nc.dram_tensor("emb", [NPC, D2], f32, kind="ExternalOutput").ap()

    xshb = nc.dram_tensor("xshb", [NPC, D1], bf).ap()
    xfull = nc.dram_tensor("xfull", [NF, D1], bf).ap()
    h1sh = nc.dram_tensor("h1sh", [NPC, D2], bf).ap()
    h1full = nc.dram_tensor("h1full", [NF, D2], bf).ap()
    h2sh = nc.dram_tensor("h2sh", [NPC, D2], bf).ap()
    h2full = nc.dram_tensor("h2full", [NF, D2], bf).ap()
    selcache = nc.dram_tensor("selcache", [P, total_blocks * P], bf).ap()

    RG = [list(range(NCORES))]
    qctr = [0]

    with tile.TileContext(nc) as tc:
        with (
            tc.tile_pool(name="const", bufs=1) as const,
            tc.tile_pool(name="selp", bufs=3) as selp,
            tc.tile_pool(name="epi", bufs=3) as epi,
        ):
            # ---- resident constants ----
            dloc_t = const.tile([P, total_blocks], bf)
            nc.sync.dma_start(out=dloc_t[:], in_=dloc_e[:, :])
            nrm_t = const.tile([P, total_blocks], bf)
            nc.sync.dma_start(out=nrm_t[:], in_=nrm_e[:, :])
            W1_t = const.tile([P, (D1 // P) * D2], bf)
            nc.sync.dma_start(out=W1_t[:], in_=W1d_e[:, :])
            W2_t = const.tile([P, (D2 // P) * D2], bf)
            nc.sync.dma_start(out=W2_t[:], in_=W2d_e[:, :])
            W3_t = []
            for ih in range(D2 // P):
                w3 = const.tile([P, DO], bf, tag=f"w3_{ih}")
                nc.sync.dma_start(out=w3[:], in_=W3d_e[ih * P:(ih + 1) * P, :])
                W3_t.append(w3)
            sbn_t = const.tile([P, 4 * nh], f32)
            nc.sync.dma_start(out=sbn_t[:], in_=sbn_e[:, :])
            b3c_t = const.tile([P, 1], f32)
            nc.sync.dma_start(out=b3c_t[:], in_=b3c_e[:, :])
            J_t = const.tile([P, P], bf)
            nc.sync.dma_start(out=J_t[:], in_=J_e[:, :])
            id_t = const.tile([P, P], bf)
            nc.sync.dma_start(out=id_t[:], in_=id_e[:, :])

            # quarter bounds (tile-aligned, match host numbering)
            qb = [(NT * a // 4) * P for a in range(5)]
            qbase_rows = [0] * 5
            for a in range(4):
                qbase_rows[a + 1] = qbase_rows[a] + NCORES * (qb[a + 1] - qb[a])

            def ag_chunked(sh, full):
                for a in range(4):
                    r0, r1 = qb[a], qb[a + 1]
                    if r1 <= r0:
                        continue
                    nc.gpsimd.collective_compute(
                        "AllGather", mybir.AluOpType.bypass, replica_groups=RG,
                        ins=[sh[r0:r1, :].opt()],
                        outs=[full[qbase_rows[a]:qbase_rows[a + 1], :].opt()],
                    )

            # ---- x allgather ----
            nc.sync.dma_start(out=xshb[:, :], in_=xsh_e[:, :])
            ag_chunked(xshb, xfull)

            def build_sel(t):
                ktot = blk_base[t][NCH] - blk_base[t][0]
                b0 = blk_base[t][0]
                sel = selp.tile([P, max_ktot * P], bf, tag="sel")
                sel3 = sel[:, :ktot * P].rearrange("p (k j) -> p k j", k=ktot)
                nc.vector.tensor_tensor(
                    out=sel3,
                    in0=dloc_t[:, b0:b0 + ktot].unsqueeze(2).broadcast_to([P, ktot, P]),
                    in1=J_t[:].unsqueeze(1).broadcast_to([P, ktot, P]),
                    op=mybir.AluOpType.is_equal,
                ).annotate("sel_eq")
                nc.vector.tensor_tensor(
                    out=sel3, in0=sel3,
                    in1=nrm_t[:, b0:b0 + ktot].unsqueeze(2).broadcast_to([P, ktot, P]),
                    op=mybir.AluOpType.mult,
                ).annotate("sel_mul")
                nc.sync.dma_start(
                    out=selcache[:, b0 * P:(b0 + ktot) * P],
                    in_=sel[:, :ktot * P]).annotate("sel_wr")
                return sel

            def stream_sel(t):
                ktot = blk_base[t][NCH] - blk_base[t][0]
                b0 = blk_base[t][0]
                sel = selp.tile([P, max_ktot * P], bf, tag="sel")
                nc.sync.dma_start(
                    out=sel[:, :ktot * P],
                    in_=selcache[:, b0 * P:(b0 + ktot) * P]).annotate("sel_rd")
                return sel

            def gather_group(g, table, D, tag, stream):
                """One dma_gather per chunk for the whole tile group."""
                cg = gcols[g]
                ixt = stream.tile([P, max_gcols], mybir.dt.int16, tag=f"ix{tag}")
                nc.sync.dma_start(
                    out=ixt[:, :cg],
                    in_=idxw_e[:, gcol8[(g, 0)]:gcol8[(g, 0)] + cg],
                ).annotate(f"idx_{tag}")
                gts = []
                for c in range(NCH):
                    nb = gnum[(g, c)]
                    if nb == 0:
                        gts.append(None)
                        continue
                    gt = stream.tile([P, gmax[c] * D], bf, tag=f"g{tag}{c}")
                    g3 = gt[:, :nb * D].rearrange("p (k d) -> p k d", d=D)
                    rows = min(VCH, NF - c * VCH)
                    lb = gcol8[(g, c)] - gcol8[(g, 0)]
                    nc.gpsimd.dma_gather(
                        out_ap=g3,
                        in_ap=table[c * VCH:c * VCH + rows, :],
                        idxs_ap=ixt[:, lb:lb + nb * 8],
                        num_idxs=nb * P, num_idxs_reg=nb * P, elem_size=D,
                        queue_num=qctr[0] % NQ, single_packet=False,
                    ).annotate(f"gth_{tag}")
                    qctr[0] += 1
                    gts.append(gt)
                return gts

            def agg(ps_s, sel, gts, t, g, D):
                ktot = blk_base[t][NCH] - blk_base[t][0]
                bi = 0
                for c in range(NCH):
                    kc = KPAD[t][c]
                    sub = gsub[(t, c)]
                    for k in range(kc):
                        nc.tensor.matmul(
                            out=ps_s[:],
                            lhsT=sel[:, bi * P:(bi + 1) * P],
                            rhs=gts[c][:, (sub + k) * D:(sub + k + 1) * D],
                            start=(bi == 0), stop=(bi == ktot - 1),
                        ).annotate("aggmm")
                        bi += 1

            # ================= Layer 1 =================
            with (
                tc.tile_pool(name="ps1", bufs=2, space="PSUM") as ps,
                tc.tile_pool(name="ps1e", bufs=1, space="PSUM") as pse,
                tc.tile_pool(name="stream1", bufs=2) as stream,
            ):
                for g in range(NG):
                    gts = gather_group(g, xfull, D1, "a", stream)
                    for t in range(g * GT, min((g + 1) * GT, NT)):
                        sel = build_sel(t)
                        ps_s = ps.tile([P, D1], f32, tag="s")
                        agg(ps_s, sel, gts, t, g, D1)
                        s_bf = epi.tile([P, D1], bf, tag="sbf")
                        nc.vector.tensor_copy(out=s_bf[:], in_=ps_s[:])
                        ps_t = pse.tile([P, D1], bf, tag="st")
                        nc.tensor.transpose(out=ps_t[:], in_=s_bf[:], identity=id_t[:])
                        sT = epi.tile([P, D1], bf, tag="sT")
                        nc.vector.tensor_copy(out=sT[:], in_=ps_t[:])
                        hT = epi.tile([P, D2], bf, tag="hT")
                        for oh in range(nh):
                            ps_h = pse.tile([P, P], f32, tag=f"h{oh}")
                            nc.tensor.matmul(
                                out=ps_h[:], lhsT=W1_t[:, oh * P:(oh + 1) * P],
                                rhs=sT[:], start=True, stop=True,
                            )
                            nc.scalar.activation(
                                out=hT[:, oh * P:(oh + 1) * P], in_=ps_h[:],
                                func=mybir.ActivationFunctionType.Relu,
                                scale=sbn_t[:, 4 * oh:4 * oh + 1],
                                bias=sbn_t[:, 4 * oh + 1:4 * oh + 2],
                            )
                        ps_nm = pse.tile([P, D2], bf, tag="nm")
                        for oh in range(nh):
                            nc.tensor.transpose(
                                out=ps_nm[:, oh * P:(oh + 1) * P],
                                in_=hT[:, oh * P:(oh + 1) * P], identity=id_t[:])
                        h_nm = epi.tile([P, D2], bf, tag="hnm")
                        nc.vector.tensor_copy(out=h_nm[:], in_=ps_nm[:])
                        nc.sync.dma_start(out=h1sh[t * P:(t + 1) * P, :], in_=h_nm[:])

            ag_chunked(h1sh, h1full)

            # ================= Layer 2 =================
            with (
                tc.tile_pool(name="ps2", bufs=2, space="PSUM") as ps,
                tc.tile_pool(name="ps2e", bufs=1, space="PSUM") as pse,
                tc.tile_pool(name="stream2", bufs=2) as stream,
            ):
                for g in range(NG):
                    gts = gather_group(g, h1full, D2, "b", stream)
                    for t in range(g * GT, min((g + 1) * GT, NT)):
                        sel = stream_sel(t)
                        ps_s = ps.tile([P, D2], f32, tag="s")
                        agg(ps_s, sel, gts, t, g, D2)
                        s_bf = epi.tile([P, D2], bf, tag="sbf2")
                        nc.vector.tensor_copy(out=s_bf[:], in_=ps_s[:])
                        ps_t = pse.tile([P, D2], bf, tag="st")
                        for ih in range(D2 // P):
                            nc.tensor.transpose(
                                out=ps_t[:, ih * P:(ih + 1) * P],
                                in_=s_bf[:, ih * P:(ih + 1) * P], identity=id_t[:])
                        sT = epi.tile([P, D2], bf, tag="sT2")
                        nc.vector.tensor_copy(out=sT[:], in_=ps_t[:])
                        hT = epi.tile([P, D2], bf, tag="hT2")
                        for oh in range(nh):
                            ps_h = pse.tile([P, P], f32, tag=f"h{oh}")
                            for ih in range(D2 // P):
                                nc.tensor.matmul(
                                    out=ps_h[:],
                                    lhsT=W2_t[:, ih * D2 + oh * P: ih * D2 + (oh + 1) * P],
                                    rhs=sT[:, ih * P:(ih + 1) * P],
                                    start=(ih == 0), stop=(ih == D2 // P - 1),
                                )
                            nc.scalar.activation(
                                out=hT[:, oh * P:(oh + 1) * P], in_=ps_h[:],
                                func=mybir.ActivationFunctionType.Relu,
                                scale=sbn_t[:, 4 * oh + 2:4 * oh + 3],
                                bias=sbn_t[:, 4 * oh + 3:4 * oh + 4],
                            )
                        ps_nm = pse.tile([P, D2], bf, tag="nm")
                        for oh in range(nh):
                            nc.tensor.transpose(
                                out=ps_nm[:, oh * P:(oh + 1) * P],
                                in_=hT[:, oh * P:(oh + 1) * P], identity=id_t[:])
                        h_nm = epi.tile([P, D2], bf, tag="hnm2")
                        nc.vector.tensor_copy(out=h_nm[:], in_=ps_nm[:])
                        nc.sync.dma_start(out=h2sh[t * P:(t + 1) * P, :], in_=h_nm[:])
                        emb_sb = epi.tile([P, D2], f32, tag="embsb")
                        nc.vector.tensor_copy(out=emb_sb[:], in_=h_nm[:])
                        nc.sync.dma_start(out=emb_e[t * P:(t + 1) * P, :], in_=emb_sb[:])

            ag_chunked(h2sh, h2full)

            # ================= Layer 3 =================
            with (
                tc.tile_pool(name="ps3", bufs=2, space="PSUM") as ps,
                tc.tile_pool(name="ps3e", bufs=1, space="PSUM") as pse,
                tc.tile_pool(name="stream3", bufs=2) as stream,
            ):
                for g in range(NG):
                    gts = gather_group(g, h2full, D2, "c", stream)
                    for t in range(g * GT, min((g + 1) * GT, NT)):
                        sel = stream_sel(t)
                        ps_s = ps.tile([P, D2], f32, tag="s3")
                        agg(ps_s, sel, gts, t, g, D2)
                        s_bf = epi.tile([P, D2], bf, tag="sbf3")
                        nc.vector.tensor_copy(out=s_bf[:], in_=ps_s[:])
                        ps_t = pse.tile([P, D2], bf, tag="st3")
                        for ih in range(D2 // P):
                            nc.tensor.transpose(
                                out=ps_t[:, ih * P:(ih + 1) * P],
                                in_=s_bf[:, ih * P:(ih + 1) * P], identity=id_t[:])
                        sT = epi.tile([P, D2], bf, tag="sT3")
                        nc.vector.tensor_copy(out=sT[:], in_=ps_t[:])
                        ps_o = pse.tile([P, P], f32, tag="o")
                        for ih in range(D2 // P):
                            nc.tensor.matmul(
                                out=ps_o[:DO, :],
                                lhsT=W3_t[ih][:],
                                rhs=sT[:, ih * P:(ih + 1) * P],
                                start=(ih == 0), stop=(ih == D2 // P - 1),
                            )
                        o_bf = epi.tile([P, P], bf, tag="obf")
                        nc.vector.tensor_tensor(
                            out=o_bf[:DO, :], in0=ps_o[:DO, :],
                            in1=b3c_t[:DO, :1].to_broadcast([DO, P]),
                            op=mybir.AluOpType.add)
                        ps_on = pse.tile([P, DO], bf, tag="on")
                        nc.tensor.transpose(
                            out=ps_on[:], in_=o_bf[:DO, :], identity=id_t[:DO, :DO])
                        o_sb = epi.tile([P, DO], f32, tag="osb")
                        nc.vector.tensor_copy(out=o_sb[:], in_=ps_on[:])
                        mx = epi.tile([P, 1], f32, tag="mx")
                        nc.vector.tensor_reduce(
                            out=mx[:], in_=o_sb[:], axis=mybir.AxisListType.X,
                            op=mybir.AluOpType.max)
                        nmx = epi.tile([P, 1], f32, tag="nmx")
                        nc.vector.tensor_scalar_mul(out=nmx[:], in0=mx[:], scalar1=-1.0)
                        eo = epi.tile([P, DO], f32, tag="eo")
                        sm = epi.tile([P, 1], f32, tag="sm")
                        nc.scalar.activation(
                            out=eo[:], in_=o_sb[:],
                            func=mybir.ActivationFunctionType.Exp,
                            bias=nmx[:, :1], scale=1.0, accum_out=sm[:])
                        lnsm = epi.tile([P, 1], f32, tag="lnsm")
                        nc.scalar.activation(
                            out=lnsm[:], in_=sm[:],
                            func=mybir.ActivationFunctionType.Ln)
                        lse = epi.tile([P, 1], f32, tag="lse")
                        nc.vector.tensor_tensor(
                            out=lse[:], in0=mx[:], in1=lnsm[:],
                            op=mybir.AluOpType.add)
                        of = epi.tile([P, DO], f32, tag="of")
                        nc.vector.tensor_tensor(
                            out=of[:], in0=o_sb[:],
                            in1=lse[:].to_broadcast([P, DO]),
                            op=mybir.AluOpType.subtract)
                        nc.sync.dma_start(out=outp_e[t * P:(t + 1) * P, :], in_=of[:])

    nc.compile()
    return nc


_CACHE = {}


def _get_program(dims):
    key = (dims["N"], dims["NPC"], dims["NT"], dims["D1"], dims["D2"],
           dims["DO"], dims["NCH"], dims["KPAD"], dims.get("debug", 0))
    if key not in _CACHE:
        _CACHE[key] = _build(dims)
    return _CACHE[key]


def run_impl(inputs, trace=False):
    in_maps, dims = _host_prep(**inputs)
    nc = _get_program(dims)
    res = run_bass_kernel_spmd(nc, in_maps, core_ids=list(range(NCORES)),
                               trace=trace)
    N = dims["N"]
    out = np.concatenate([r["outp"] for r in res.results], axis=0)[:N]
    emb = np.concatenate([r["emb"] for r in res.results], axis=0)[:N]
    return (out.astype(np.float32), emb.astype(np.float32)), res


def kernel(**inputs):
    (out, emb), _ = run_impl(inputs, trace=False)
    return out, emb
